# revision 26
# baseline (speedup 1.0000x reference)
"""DLRM (bottom MLP + embedding gather + pairwise interaction + top MLP)
on 8 Trainium2 NeuronCores, batch-parallel (512 samples/core), embedding
tables replicated. All sharding/marshalling on host; one SPMD Bass program.

Numerics: dense path (bottom MLP, top MLP x-part/L2/L3) in float32r
matmuls (full-rate fp32, ~tf32 rounding, ~1e-4 rel); interaction path
(embedding gather, grams, top-L1 Z-part) in bf16.

Layout: embeddings gathered bf16 [sample-part, table*d]; PE-transposed
per table ([128,64]->[64,128], 8 sharing a [64,1024] PSUM tile drained
with one contiguous copy) into feature-major tw [64(d), feat*128]; per-
sample gram matmuls read strided [64, 27] views, 4-way PSUM-quadrant
packed; Z drained into j-partition zbuf i-major (contiguous writes,
so the repack rhs streams 64B runs at full PE rate); repack ON THE PE
via selector matmuls that compact the 351 lower-triangle pairs into 3
dense 128-row K-chunks (row t = i(i-1)/2 + j - 128g), so top-L1 does
only 12 z-matmuls and tw0z is 351 rows, with auto-zero pad rows.
The HAM clock gate needs ~3.4us sustained matmul activity for 2.4GHz
and PE-mode transposes do NOT count as activity, so the warm-up is
~3.6us, dummy matmuls are sprinkled between transpose groups, and the
wave-3-drain tail is bridged with dummies + top-L1 x-part matmuls.
"""
import numpy as np

B = 4096
NCORES = 8
BC = B // NCORES          # 512 samples per core
NT = 26                   # embedding tables
V = 100000                # vocab per table
D = 64                    # embedding dim
NI = NT + 1               # 27 interaction features
M_DEN = 13
H0, H1 = 512, 256         # bottom MLP hidden (13->512->256->64)
T0, T1 = 512, 256         # top MLP hidden (415->512->256->1)
NP = NI * (NI - 1) // 2   # 351 lower-triangle pairs
NZC = 3                   # dense K-chunks of 128 pair-rows (384 >= 351)
NW = 4                    # waves (one per 128-sample block)

_CACHE = {}


def _repack_plan():
    """(gch, i, start, stop) per selector matmul: chunk gch accumulates
    pair-rows t = i(i-1)/2 + j (j < i) that fall in [128g, 128(g+1))."""
    plan = []  # [(gch, i, slot)]
    for gch in range(NZC):
        lo, hi = 128 * gch, 128 * (gch + 1)
        for i in range(1, NI):
            t0, t1 = i * (i - 1) // 2, i * (i - 1) // 2 + i
            if t1 > lo and t0 < hi:
                plan.append((gch, i))
    return plan


def _build_program(taps=False):
    import concourse.bass as bass
    import concourse.bacc as bacc
    import concourse.mybir as mybir
    import concourse.tile as tile
    from concourse.masks import make_identity
    from contextlib import ExitStack

    dt = mybir.dt
    f32, bf16, i32 = dt.float32, dt.bfloat16, dt.int32
    f32r = dt.float32r  # fp32 @ 1cyc/col on PE (N>=256), ~tf32 rounding

    nc = bacc.Bacc("TRN2", target_bir_lowering=False, debug=False,
                   num_devices=NCORES)

    def din(name, shape, dtype=f32):
        return nc.dram_tensor(name, shape, dtype, kind="ExternalInput").ap()

    plan = _repack_plan()
    NSEL = len(plan)  # 28 selector matrices

    emb = din("emb", [NT * V, D], bf16)
    offs_d = din("offs", [128, NW * NT], i32)
    # merged const blobs (fewer DMAs):
    # wb13 = [bw0 | xT] on 13 partitions
    wb13_d = din("wb13", [M_DEN, H0 + BC], f32r)
    # wblob f32 [128, 2320]:
    #   bb0[0:4] bw1[4:1028] bb1[1028:1030] bw2x2[1030:1286] bb2[1286:1287]
    #   tb0[1287:1291] tw1[1291:2315] tb1[2315:2317] tw2[2317:2319]
    #   tb2 at [0, 2319]; bw2/bb2 are column-duplicated so the last
    #   bottom-MLP layer emits xe on BOTH partition halves (M=128) --
    #   the pair-gram B-half x feature then needs no partition-shift DMA
    wblob_d = din("wblob", [128, 2320], f32r)
    tw0x = din("tw0x", [D, T0], f32r)          # [64, 512]
    tw0z = din("tw0z", [128, NZC * T0], bf16)  # 3 dense K-chunks [128, 512]
    selm_d = din("selm", [NI, (NSEL + 1) * 128], bf16)  # repack sels + zero
    out_d = nc.dram_tensor("outT", [1, BC], f32, kind="ExternalOutput").ap()
    tap_d = {}
    if taps:
        for nm, shape, dty in [
                ("dbg_xe", [D, BC], f32), ("dbg_g0", [128, NT * D], bf16),
                ("dbg_tw0", [64, 64 * NI], bf16),
                ("dbg_zbuf", [NI, 4 * 128 * NI], bf16),
                ("dbg_zdense", [128, NZC * BC], bf16),
                ("dbg_o1", [128, 2048], f32)]:
            tap_d[nm] = nc.dram_tensor(nm, shape, dty,
                                       kind="ExternalOutput").ap()

    with tile.TileContext(nc) as tc:
        with ExitStack() as ctx:
            cp = ctx.enter_context(tc.tile_pool(name="const", bufs=1))
            gp = ctx.enter_context(tc.tile_pool(name="gath", bufs=4))
            tp = ctx.enter_context(tc.tile_pool(name="tall", bufs=3))
            zp = ctx.enter_context(tc.tile_pool(name="zbuf", bufs=1))
            hp = ctx.enter_context(tc.tile_pool(name="acts", bufs=1))
            pt = ctx.enter_context(
                tc.tile_pool(name="ps_t", bufs=2, space="PSUM"))
            pz = ctx.enter_context(
                tc.tile_pool(name="ps_z", bufs=2, space="PSUM"))
            pm = ctx.enter_context(
                tc.tile_pool(name="ps_m", bufs=2, space="PSUM"))

            def const_tile(ap, shape, tag=None):
                t = cp.tile(shape, ap.dtype, tag=tag or ap.tensor.name)
                nc.sync.dma_start(t[:], ap)
                return t

            offs = const_tile(offs_d, [128, NW * NT])
            wb13 = const_tile(wb13_d, [M_DEN, H0 + BC])
            ident = cp.tile([128, 128], bf16, tag="ident")
            make_identity(nc, ident[:])
            wblob = const_tile(wblob_d, [128, 2320])
            w_t0x = const_tile(tw0x, [D, T0])
            w_t0z = const_tile(tw0z, [128, NZC * T0])
            selm = const_tile(selm_d, [NI, (NSEL + 1) * 128])
            w_bw0 = wb13[:, 0:H0]
            xT = wb13[:, H0:H0 + BC]
            w_bw1 = wblob[:, 4:1028]
            w_bw2 = wblob[:, 1030:1286]
            w_tw1 = wblob[:, 1291:2315]
            w_tw2 = wblob[:, 2317:2319]
            # biases: plain-f32 views of the f32r blob
            w_bb0 = wblob[:, 0:4].bitcast(f32)
            w_bb1 = wblob[:, 1028:1030].bitcast(f32)
            w_bb2 = wblob[:, 1286:1287].bitcast(f32)
            w_tb0 = wblob[:, 1287:1291].bitcast(f32)
            w_tb1 = wblob[:, 2315:2317].bitcast(f32)
            w_tb2 = wblob[0:1, 2319:2320].bitcast(f32)

            RELU = mybir.ActivationFunctionType.Relu
            IDENT = mybir.ActivationFunctionType.Identity
            mm = nc.tensor.matmul
            mmr = mm  # operands are f32r-typed tiles already

            # gpsimd copies are slow (~2us fixed) and cannot read PSUM:
            # all marshalling copies alternate vector/scalar
            _ps_engines = [nc.vector.tensor_copy, nc.scalar.copy]
            _ci = [0]

            def rcopy(dst, src):
                _ps_engines[_ci[0] % 2](dst, src)
                _ci[0] += 1

            # zdense: 3 K-chunk tiles, fully written by the selector-
            # matmul repack (pad rows come out zero), no memset needed
            zdense = [zp.tile([128, BC], bf16, tag=f"zd{g2}",
                              name=f"zd{g2}")
                      for g2 in range(NZC)]

            # PE warm-up: dummy matmuls on the identity so the HAM
            # clock-gate is at 8/8 before the bottom MLP issues (needs
            # ~3.4us of SUSTAINED matmul activity; transposes don't count)
            wps = pm.tile([128, BC], f32, tag="mlp")
            for _ in range(34):
                mm(wps[0:64, 0:128], ident[0:64, 0:64],
                   ident[0:64, 0:128], start=True, stop=True)

            # issue all 4 wave gathers up front (gpsimd DGE runs ahead)
            gtiles = []
            for w in range(NW):
                g = gp.tile([128, NT * D], bf16, tag="g")
                nc.gpsimd.indirect_dma_start(
                    out=g[:],
                    out_offset=None,
                    in_=emb,
                    in_offset=bass.IndirectOffsetOnAxis(
                        ap=offs[:, NT * w:NT * (w + 1)], axis=0),
                )
                gtiles.append(g)

            # ---- wave marshalling: quad-gram operand tiles ----
            # Grams are batched 4 samples per matmul: K=128 block-diag
            # (pair A-sample on partitions 0-63, B-sample on 64-127,
            # zero elsewhere) x M=108 (two pairs side by side; cross-
            # pair same-half products land in unused out positions).
            # tw128 layout [128, (a 2, i 27, p 64)]: col = 1728a+64i+p;
            # pair p = wave samples (p, p+64). A-halves drain straight
            # from the transpose PSUM (lane-aligned); B-halves stage in
            # twtB and shift partitions 0-63 -> 64-127 via per-group
            # SBUF->SBUF DMAs. Zero quadrants memset once per buffer.
            # All units are WOVEN into the surrounding matmul stream:
            # PE transposes do NOT register as HAM clock-gate activity,
            # and neither do the tiny matmuls, so gaps must stay short.
            tw128s = [zp.tile([128, 4096], bf16, tag=f"tw{k}",
                              name=f"tw128_{k}") for k in range(3)]
            for k in range(3):
                nc.vector.memset(tw128s[k][0:64, 1728:4096], 0.0)
                nc.vector.memset(tw128s[k][64:128, 0:2048], 0.0)
                nc.vector.memset(tw128s[k][64:128, 3776:4096], 0.0)

            def twtile(w):
                return tw128s[w % 3]

            def marshal_units(w, with_x):
                g = gtiles[w]
                tw = twtile(w)
                twtB = tp.tile([64, 64 * NI], bf16, tag="tB",
                               name=f"twtB{w}")
                units = []
                if with_x:
                    # x as interaction feature 0 (cast f32 -> bf16);
                    # xe lives on both partition halves, so both copies
                    # are lane-aligned
                    units.append(lambda tw=tw, w=w: rcopy(
                        tw[0:64, 0:64],
                        xe[0:64, 128 * w:128 * w + 64].bitcast(f32)))
                    units.append(lambda tw=tw, w=w: rcopy(
                        tw[64:128, 2048:2112],
                        xe[64:128, 128 * w + 64:128 * (w + 1)].bitcast(f32)))
                for grp in range(4):
                    lo = 8 * grp
                    hi = min(lo + 8, NT)
                    pst = pt.tile([64, 1024], bf16, tag="tr",
                                  name=f"tr{w}{grp}")
                    for u in range(lo, hi):
                        units.append(
                            lambda pst=pst, u=u, lo=lo, g=g:
                            nc.tensor.transpose(
                                pst[:, 128 * (u - lo):128 * (u - lo + 1)],
                                g[:, 64 * u:64 * (u + 1)], ident[:]))
                    pstv = pst[:].rearrange("d (u s) -> d u s", s=128)
                    nu = hi - lo
                    # A-samples (sl 0-63) -> tw128 top half, in place
                    units.append(
                        lambda pstv=pstv, lo=lo, nu=nu, tw=tw:
                        rcopy(tw[0:64, 64 * (1 + lo):64 * (1 + lo + nu)
                                 ].rearrange("d (u p) -> d u p", p=64),
                              pstv[:, 0:nu, 0:64]))
                    # B-samples (sl 64-127) -> twtB staging
                    units.append(
                        lambda pstv=pstv, lo=lo, nu=nu, twtB=twtB:
                        rcopy(twtB[:, 64 * (1 + lo):64 * (1 + lo + nu)
                                   ].rearrange("d (u p) -> d u p", p=64),
                              pstv[:, 0:nu, 64:128]))
                    # partition-shift B into tw128 bottom half (covers
                    # the x slot too for grp 0)
                    blo = 64 * (1 + lo)
                    bhi = 64 * (1 + hi)
                    units.append(
                        lambda twtB=twtB, tw=tw, blo=blo, bhi=bhi:
                        nc.sync.dma_start(tw[64:128, 2048 + blo:2048 + bhi],
                                          twtB[:, blo:bhi]))
                return units, twtB

            u0, twtB0 = marshal_units(0, with_x=False)

            def drip(n):
                for _ in range(min(n, len(u0))):
                    u0.pop(0)()

            # ---- bottom MLP: h0 = relu(x @ bw0.T + bb0), wave-0
            # transposes woven between the matmuls ----
            h0 = hp.tile([128, 2048], f32r, tag="h0")
            for m in range(4):
                ps = pm.tile([128, BC], f32, tag="mlp")
                mmr(ps[:], w_bw0[:, 128 * m:128 * (m + 1)], xT[:],
                    start=True, stop=True)
                drip(2)
                nc.scalar.activation(h0[:, 512 * m:512 * (m + 1)], ps[:],
                                     RELU, bias=w_bb0[:, m:m + 1])
            # ---- h1 = relu(h0 @ bw1.T + bb1): K=512 (4 chunks), M=256 ----
            h1 = hp.tile([128, 1024], f32r, tag="h1")
            for n in range(2):
                ps = pm.tile([128, BC], f32, tag="mlp")
                for k in range(4):
                    mmr(ps[:], w_bw1[:, 256 * k + 128 * n:256 * k + 128 * (n + 1)],
                        h0[:, 512 * k:512 * (k + 1)],
                        start=(k == 0), stop=(k == 3))
                    drip(2)
                nc.scalar.activation(h1[:, 512 * n:512 * (n + 1)], ps[:],
                                     RELU, bias=w_bb1[:, n:n + 1])
            # ---- xe = h1 @ bw2.T + bb2: K=256 (2 chunks), M=128
            # (xe duplicated on both partition halves) ----
            xe = hp.tile([128, BC], f32r, tag="xe")
            psx = pm.tile([128, BC], f32, tag="mlp")
            for k in range(2):
                mmr(psx[:], w_bw2[:, 128 * k:128 * (k + 1)],
                    h1[:, 512 * k:512 * (k + 1)], start=(k == 0), stop=(k == 1))
                drip(3)
            drip(38)
            nc.scalar.activation(xe[:], psx[:], IDENT,
                                 bias=w_bb2[:, 0:1])
            if taps:
                nc.sync.dma_start(tap_d["dbg_xe"], xe[0:D, :].bitcast(f32))
            # wave-0 x feature (needs xe): both halves lane-aligned
            rcopy(twtile(0)[0:64, 0:64], xe[0:64, 0:64].bitcast(f32))
            rcopy(twtile(0)[64:128, 2048:2112],
                  xe[64:128, 64:128].bitcast(f32))

            # ---- pair-gram matmuls, per 128-sample wave: 64 matmuls of
            # [128,54]x[128,54] (2 samples each, K=128 block-diag), 2-up
            # PSUM-packed via 64-col tiling; next wave's marshalling
            # units woven 1-per-matmul. Pair p = samples (p, p+64); the
            # pair operand [128, (a 2: +1728, i 27: +64)] merges to a
            # single [64-stride, 54] free dim (BIR rhs constraint).
            zbuf = zp.tile([NI, 4 * 128 * NI], bf16, tag="zbuf")  # [27,13824]
            for w in range(NW):
                tw = twtile(w)
                v = tw[:].rearrange("d (a i p) -> d p a i", a=2, i=32)
                uq = (marshal_units(w + 1, with_x=True)[0]
                      if w + 1 < NW else [])
                for tau in range(2):
                    zq = pz.tile([128, 1024], f32, tag="z",
                                 name=f"zq{w}{tau}")
                    for half in range(2):
                        for q in range(16):
                            p = 32 * tau + 16 * half + q
                            op = v[:, p, :, :]  # [128, 2, 32] -> [128, 64]
                            mm(zq[64 * half:64 * half + 64,
                                  64 * q:64 * q + 64], op, op,
                               start=True, stop=True,
                               tile_position=(0, 64 * half))
                            for _ in range(2):
                                if uq:
                                    uq.pop(0)()
                    # drain the 4 valid diag-block sets (half t, a) to
                    # zbuf i-major: sample sl = 32tau + 16t + q + 64a,
                    # c = tau + 2a, kw = 16t + q; symmetric relabel puts
                    # j on partitions
                    for t in range(2):
                        for a in range(2):
                            c = tau + 2 * a
                            src = zq[64 * t + 32 * a:64 * t + 32 * a + NI,
                                     :].rearrange(
                                "j (q v2) -> j v2 q", v2=64)[
                                :, 32 * a:32 * a + NI, :]
                            dst = zbuf[
                                :, 3456 * c + 864 * w:3456 * c + 864 * (w + 1)
                            ].rearrange("j (i k) -> j i k", k=32)[
                                :, :, 16 * t:16 * t + 16]
                            rcopy(dst, src)
                while uq:
                    uq.pop(0)()

            if taps:
                nc.sync.dma_start(tap_d["dbg_zbuf"], zbuf[:])

            # keep the PE warm across the wave-3 drain tail, and overlap
            # it with the top-L1 x-part for m=0,1 (no z dependency) into
            # a freed gram-PSUM tile used as the L1 accumulator
            wdum = pm.tile([128, BC], f32, tag="mlp")
            for _ in range(30):
                mm(wdum[0:64, 0:128], ident[0:64, 0:64],
                   ident[0:64, 0:128], start=True, stop=True)
            xrhs = xe[0:64, :].rearrange("d (w c j) -> d c w j", c=4, j=32)
            pl01 = pz.tile([128, 1024], f32, tag="z")
            pl23 = pz.tile([128, 1024], f32, tag="z")
            for m in range(4):
                pl = (pl01, pl23)[m // 2][:, 512 * (m % 2):512 * (m % 2 + 1)]
                mmr(pl, w_t0x[:, 128 * m:128 * (m + 1)], xrhs,
                    start=True, stop=False)

            # ---- repack Z into 3 dense K-chunks of lower-tri pair rows:
            # row t = i(i-1)/2 + j - 128*gch. Selector matmuls on the PE:
            # sel[j, m] = (j < i and t == 128g + m), so each mm lands i's
            # valid j-rows at their pair positions, zero elsewhere; one
            # contiguous [128,512] f32->bf16 drain per chunk.
            # rhs view per i: [27(j), (c, w, kw)] -- 32-element (64B)
            # contiguous runs, full-rate PE streaming; N order (c, w, kw)
            # matches the zdense/top-L1 sample order 128c + 32w + kw
            zr = zbuf[:].rearrange("j (c w i k) -> j c w i k", c=4, w=NW,
                                   i=NI)

            def repack_chunk(gch):
                psr = pm.tile([128, BC], f32, tag="mlp", name=f"psr{gch}")
                sub = [(si, i) for si, (gc, i) in enumerate(plan)
                       if gc == gch]
                for nn, (si, i) in enumerate(sub):
                    mm(psr[:], selm[:, 128 * si:128 * (si + 1)],
                       zr[:, :, :, i, :],
                       start=(nn == 0), stop=(nn == len(sub) - 1))
                rcopy(zdense[gch][:], psr[:])

            def l1z(gch):
                for m in range(4):
                    pl = (pl01, pl23)[m // 2][
                        :, 512 * (m % 2):512 * (m % 2 + 1)]
                    mm(pl,
                       w_t0z[:, T0 * gch + 128 * m:T0 * gch + 128 * (m + 1)],
                       zdense[gch][:],
                       start=False, stop=(gch == NZC - 1))

            repack_chunk(0)
            repack_chunk(1)
            l1z(0)
            repack_chunk(2)
            l1z(1)
            l1z(2)

            if taps:
                for _g in range(NZC):
                    nc.sync.dma_start(
                        tap_d["dbg_zdense"].rearrange(
                            "p (g s) -> p g s", g=NZC)[:, _g, :],
                        zdense[_g][:])
            # ---- top L1 activations (x-parts + z accumulated above) ----
            o1 = hp.tile([128, 2048], f32r, tag="h0")
            for m in range(4):
                pl = (pl01, pl23)[m // 2][:, 512 * (m % 2):512 * (m % 2 + 1)]
                nc.scalar.activation(o1[:, 512 * m:512 * (m + 1)], pl,
                                     RELU, bias=w_tb0[:, m:m + 1])
            if taps:
                nc.sync.dma_start(tap_d["dbg_o1"], o1[:])
            # ---- top L2: K=512 (4 chunks), M=256 ----
            o2 = hp.tile([128, 1024], f32r, tag="h1")
            for n in range(2):
                ps = pm.tile([128, BC], f32, tag="mlp")
                for k in range(4):
                    mmr(ps[:], w_tw1[:, 256 * k + 128 * n:256 * k + 128 * (n + 1)],
                        o1[:, 512 * k:512 * (k + 1)],
                        start=(k == 0), stop=(k == 3))
                nc.scalar.activation(o2[:, 512 * n:512 * (n + 1)], ps[:],
                                     RELU, bias=w_tb1[:, n:n + 1])
            # ---- top L3: K=256 (2 chunks), M=1 ----
            osb = hp.tile([1, BC], f32, tag="osb")
            ps3 = pm.tile([128, BC], f32, tag="mlp")
            for k in range(2):
                mmr(ps3[0:1, :], w_tw2[:, k:k + 1],
                    o2[:, 512 * k:512 * (k + 1)], start=(k == 0), stop=(k == 1))
            nc.scalar.activation(osb[:], ps3[0:1, :], IDENT,
                                 bias=w_tb2[0:1, 0:1])
            nc.sync.dma_start(out_d, osb[:])
    nc.compile()
    return nc


def _host_prep(inputs):
    import ml_dtypes
    f = np.float32
    dense_x = np.asarray(inputs["dense_x"], f)
    sparse_idx = np.asarray(inputs["sparse_idx"])
    emb = np.ascontiguousarray(
        np.asarray(inputs["emb"], f).reshape(NT * V, D).astype(
            ml_dtypes.bfloat16))
    gl = (np.arange(NT, dtype=np.int64)[:, None] * V + sparse_idx).astype(
        np.int32)  # [26, 4096] global row ids

    bw0, bb0 = np.asarray(inputs["bw0"], f), np.asarray(inputs["bb0"], f)
    bw1, bb1 = np.asarray(inputs["bw1"], f), np.asarray(inputs["bb1"], f)
    bw2, bb2 = np.asarray(inputs["bw2"], f), np.asarray(inputs["bb2"], f)
    tw0, tb0 = np.asarray(inputs["tw0"], f), np.asarray(inputs["tb0"], f)
    tw1, tb1 = np.asarray(inputs["tw1"], f), np.asarray(inputs["tb1"], f)
    tw2, tb2 = np.asarray(inputs["tw2"], f), np.asarray(inputs["tb2"], f)

    def kpack(wT, nk, m):  # [K, M] -> [128, nk*m] chunk-major
        return np.ascontiguousarray(
            wT.reshape(nk, 128, m).transpose(1, 0, 2).reshape(128, nk * m))

    # top-L1 z weights: pair t = i(i-1)/2 + j (tril_indices order), one
    # row per lower-triangle pair, pad rows 351..383 zero
    wpad = np.zeros((NZC * 128, T0), f)
    wpad[:NP] = tw0[:, D:].T  # [351, 512]
    tw0z = kpack(wpad, NZC, T0).astype(ml_dtypes.bfloat16)

    # repack selectors: for (gch, i): sel[j, m] = 1 iff j < i and
    # i(i-1)/2 + j == 128*gch + m
    plan = _repack_plan()
    selm = np.zeros((NI, (len(plan) + 1) * 128), f)
    for si, (gch, i) in enumerate(plan):
        for j in range(i):
            t = i * (i - 1) // 2 + j
            m = t - 128 * gch
            if 0 <= m < 128:
                selm[j, 128 * si + m] = 1.0

    bb2p = np.zeros((128, 1), f)
    bb2p[:D, 0] = bb2
    bb2p[D:, 0] = bb2
    tb2p = np.zeros((128, 1), f)
    tb2p[0, 0] = tb2.reshape(1)[0]
    bw2d = np.ascontiguousarray(
        np.concatenate([bw2.T, bw2.T], axis=1))  # [256, 128]
    wblob = np.ascontiguousarray(np.concatenate([
        bb0.reshape(4, 128).T,                 # [128, 4]
        kpack(np.ascontiguousarray(bw1.T), 4, H1),   # [128, 1024]
        bb1.reshape(2, 128).T,                 # [128, 2]
        kpack(bw2d, 2, 128),                   # [128, 256]
        bb2p,                                  # [128, 1]
        tb0.reshape(4, 128).T,                 # [128, 4]
        kpack(np.ascontiguousarray(tw1.T), 4, T1),   # [128, 1024]
        tb1.reshape(2, 128).T,                 # [128, 2]
        kpack(np.ascontiguousarray(tw2.T), 2, 1),    # [128, 2]
        tb2p,                                  # [128, 1]
    ], axis=1))
    bw0T = np.ascontiguousarray(bw0.T)         # [13, 512]
    shared = {
        "emb": emb,
        "wblob": wblob,
        "tw0x": np.ascontiguousarray(tw0[:, :D].T),               # [64, 512]
        "tw0z": tw0z,
        "selm": selm.astype(ml_dtypes.bfloat16),
    }
    in_maps = []
    for c in range(NCORES):
        sl = gl[:, BC * c:BC * (c + 1)]  # [26, 512]
        offs = np.ascontiguousarray(
            sl.reshape(NT, NW, 128).transpose(2, 1, 0).reshape(128, NW * NT))
        m = dict(shared)
        m["offs"] = offs
        m["wb13"] = np.ascontiguousarray(np.concatenate(
            [bw0T, dense_x[BC * c:BC * (c + 1)].T], axis=1))
        in_maps.append(m)
    return in_maps


def _unpermute():
    s = np.arange(BC)
    pos = ((s % 128) // 32) * 128 + (s // 128) * 32 + (s % 32)
    return pos  # out[s] = outT[0, pos[s]]


def kernel(**inputs):
    from concourse import bass_utils
    if "nc" not in _CACHE:
        _CACHE["nc"] = _build_program()
    nc = _CACHE["nc"]
    in_maps = _host_prep(inputs)
    res = bass_utils.run_bass_kernel_spmd(nc, in_maps,
                                          core_ids=list(range(NCORES)))
    pos = _unpermute()
    out = np.empty((B, 1), np.float32)
    for c in range(NCORES):
        out[BC * c:BC * (c + 1), 0] = res.results[c]["outT"][0, pos]
    return out


# revision 27
# speedup vs baseline: 1.1910x; 1.1910x over previous
"""DLRM (bottom MLP + embedding gather + pairwise interaction + top MLP)
on 8 Trainium2 NeuronCores, batch-parallel (512 samples/core), embedding
tables replicated. All sharding/marshalling on host; one SPMD Bass program.

Numerics: dense path (bottom MLP, top MLP x-part/L2/L3) in float32r
matmuls (full-rate fp32, ~tf32 rounding, ~1e-4 rel); interaction path
(embedding gather, grams, top-L1 Z-part) in bf16.

Layout: embeddings gathered bf16 [sample-part, table*d]; PE-transposed
per table ([128,64]->[64,128], 8 sharing a [64,1024] PSUM tile drained
with one contiguous copy) into feature-major tw [64(d), feat*128]; per-
sample gram matmuls read strided [64, 27] views, 4-way PSUM-quadrant
packed; Z drained into j-partition zbuf i-major (contiguous writes,
so the repack rhs streams 64B runs at full PE rate); repack ON THE PE
via selector matmuls that compact the 351 lower-triangle pairs into 3
dense 128-row K-chunks (row t = i(i-1)/2 + j - 128g), so top-L1 does
only 12 z-matmuls and tw0z is 351 rows, with auto-zero pad rows.
The HAM clock gate needs ~3.4us sustained matmul activity for 2.4GHz
and PE-mode transposes do NOT count as activity, so the warm-up is
~3.6us, dummy matmuls are sprinkled between transpose groups, and the
wave-3-drain tail is bridged with dummies + top-L1 x-part matmuls.
"""
import numpy as np

B = 4096
NCORES = 8
BC = B // NCORES          # 512 samples per core
NT = 26                   # embedding tables
V = 100000                # vocab per table
D = 64                    # embedding dim
NI = NT + 1               # 27 interaction features
M_DEN = 13
H0, H1 = 512, 256         # bottom MLP hidden (13->512->256->64)
T0, T1 = 512, 256         # top MLP hidden (415->512->256->1)
NP = NI * (NI - 1) // 2   # 351 lower-triangle pairs
NZC = 3                   # dense K-chunks of 128 pair-rows (384 >= 351)
NW = 4                    # waves (one per 128-sample block)

_CACHE = {}


def _repack_plan():
    """(gch, i, start, stop) per selector matmul: chunk gch accumulates
    pair-rows t = i(i-1)/2 + j (j < i) that fall in [128g, 128(g+1))."""
    plan = []  # [(gch, i, slot)]
    for gch in range(NZC):
        lo, hi = 128 * gch, 128 * (gch + 1)
        for i in range(1, NI):
            t0, t1 = i * (i - 1) // 2, i * (i - 1) // 2 + i
            if t1 > lo and t0 < hi:
                plan.append((gch, i))
    return plan


def _build_program(taps=False):
    import concourse.bass as bass
    import concourse.bacc as bacc
    import concourse.mybir as mybir
    import concourse.tile as tile
    from contextlib import ExitStack

    dt = mybir.dt
    f32, bf16, i32 = dt.float32, dt.bfloat16, dt.int32
    f32r = dt.float32r  # fp32 @ 1cyc/col on PE (N>=256), ~tf32 rounding

    nc = bacc.Bacc("TRN2", target_bir_lowering=False, debug=False,
                   num_devices=NCORES)

    def din(name, shape, dtype=f32):
        return nc.dram_tensor(name, shape, dtype, kind="ExternalInput").ap()

    plan = _repack_plan()
    NSEL = len(plan)  # 28 selector matrices

    emb = din("emb", [NT * V, D], bf16)
    offs_d = din("offs", [128, NW * NT], i32)
    # merged const blobs (fewer DMAs):
    # wb13 = [bw0 | xT] on 13 partitions
    wb13_d = din("wb13", [M_DEN, H0 + BC], f32r)
    # wblob f32 [128, 2320]:
    #   bb0[0:4] bw1[4:1028] bb1[1028:1030] bw2x2[1030:1286] bb2[1286:1287]
    #   tb0[1287:1291] tw1[1291:2315] tb1[2315:2317] tw2[2317:2319]
    #   tb2 at [0, 2319]; bw2/bb2 are column-duplicated so the last
    #   bottom-MLP layer emits xe on BOTH partition halves (M=128) --
    #   the pair-gram B-half x feature then needs no partition-shift DMA
    wblob_d = din("wblob", [128, 2320], f32r)
    identc_d = din("identc", [128, 128], bf16)
    tw0x = din("tw0x", [D, T0], f32r)          # [64, 512]
    tw0z = din("tw0z", [128, NZC * T0], bf16)  # 3 dense K-chunks [128, 512]
    selm_d = din("selm", [NI, (NSEL + 1) * 128], bf16)  # repack sels + zero
    out_d = nc.dram_tensor("outT", [1, BC], f32, kind="ExternalOutput").ap()
    tap_d = {}
    if taps:
        for nm, shape, dty in [
                ("dbg_xe", [D, BC], f32), ("dbg_g0", [128, NT * D], bf16),
                ("dbg_tw0", [64, 64 * NI], bf16),
                ("dbg_zbuf", [NI, 4 * 128 * NI], bf16),
                ("dbg_zdense", [128, NZC * BC], bf16),
                ("dbg_o1", [128, 2048], f32)]:
            tap_d[nm] = nc.dram_tensor(nm, shape, dty,
                                       kind="ExternalOutput").ap()

    with tile.TileContext(nc) as tc:
        with ExitStack() as ctx:
            cp = ctx.enter_context(tc.tile_pool(name="const", bufs=1))
            gp = ctx.enter_context(tc.tile_pool(name="gath", bufs=4))
            tp = ctx.enter_context(tc.tile_pool(name="tall", bufs=3))
            zp = ctx.enter_context(tc.tile_pool(name="zbuf", bufs=1))
            hp = ctx.enter_context(tc.tile_pool(name="acts", bufs=1))
            pt = ctx.enter_context(
                tc.tile_pool(name="ps_t", bufs=2, space="PSUM"))
            pz = ctx.enter_context(
                tc.tile_pool(name="ps_z", bufs=2, space="PSUM"))
            pm = ctx.enter_context(
                tc.tile_pool(name="ps_m", bufs=2, space="PSUM"))

            def const_tile(ap, shape, tag=None):
                t = cp.tile(shape, ap.dtype, tag=tag or ap.tensor.name)
                nc.sync.dma_start(t[:], ap)
                return t

            offs = const_tile(offs_d, [128, NW * NT])
            ident = const_tile(identc_d, [128, 128])
            wb13 = const_tile(wb13_d, [M_DEN, H0 + BC])
            wblob = const_tile(wblob_d, [128, 2320])
            w_t0x = const_tile(tw0x, [D, T0])
            w_t0z = const_tile(tw0z, [128, NZC * T0])
            selm = const_tile(selm_d, [NI, (NSEL + 1) * 128])
            w_bw0 = wb13[:, 0:H0]
            xT = wb13[:, H0:H0 + BC]
            w_bw1 = wblob[:, 4:1028]
            w_bw2 = wblob[:, 1030:1286]
            w_tw1 = wblob[:, 1291:2315]
            w_tw2 = wblob[:, 2317:2319]
            # biases: plain-f32 views of the f32r blob
            w_bb0 = wblob[:, 0:4].bitcast(f32)
            w_bb1 = wblob[:, 1028:1030].bitcast(f32)
            w_bb2 = wblob[:, 1286:1287].bitcast(f32)
            w_tb0 = wblob[:, 1287:1291].bitcast(f32)
            w_tb1 = wblob[:, 2315:2317].bitcast(f32)
            w_tb2 = wblob[0:1, 2319:2320].bitcast(f32)

            RELU = mybir.ActivationFunctionType.Relu
            IDENT = mybir.ActivationFunctionType.Identity
            mm = nc.tensor.matmul
            mmr = mm  # operands are f32r-typed tiles already

            # gpsimd copies are slow (~2us fixed) and cannot read PSUM:
            # all marshalling copies alternate vector/scalar
            _ps_engines = [nc.vector.tensor_copy, nc.scalar.copy]
            _ci = [0]

            def rcopy(dst, src):
                _ps_engines[_ci[0] % 2](dst, src)
                _ci[0] += 1

            # zdense: 3 K-chunk tiles, fully written by the selector-
            # matmul repack (pad rows come out zero), no memset needed
            zdense = [zp.tile([128, BC], bf16, tag=f"zd{g2}",
                              name=f"zd{g2}")
                      for g2 in range(NZC)]

            # PE warm-up: dummy matmuls on GARBAGE bits from the very
            # first const DMA (offs) -- results are discarded, so NaNs
            # are harmless, and the PE starts ~1.5us in instead of
            # waiting ~7us for a gpsimd-built identity (HAM clock-gate
            # needs ~3.4us of sustained matmul activity)
            offsb = offs[:].bitcast(bf16)  # [128, 208] garbage
            wps = pm.tile([128, BC], f32, tag="mlp")
            for _ in range(34):
                mm(wps[0:128, 0:128], offsb[0:64, 0:128],
                   offsb[0:64, 0:128], start=True, stop=True)

            # issue all 4 wave gathers up front (gpsimd DGE runs ahead)
            gtiles = []
            for w in range(NW):
                g = gp.tile([128, NT * D], bf16, tag="g")
                nc.gpsimd.indirect_dma_start(
                    out=g[:],
                    out_offset=None,
                    in_=emb,
                    in_offset=bass.IndirectOffsetOnAxis(
                        ap=offs[:, NT * w:NT * (w + 1)], axis=0),
                )
                gtiles.append(g)

            # ---- wave marshalling: quad-gram operand tiles ----
            # Grams are batched 4 samples per matmul: K=128 block-diag
            # (pair A-sample on partitions 0-63, B-sample on 64-127,
            # zero elsewhere) x M=108 (two pairs side by side; cross-
            # pair same-half products land in unused out positions).
            # tw128 layout [128, (a 2, i 27, p 64)]: col = 1728a+64i+p;
            # pair p = wave samples (p, p+64). A-halves drain straight
            # from the transpose PSUM (lane-aligned); B-halves stage in
            # twtB and shift partitions 0-63 -> 64-127 via per-group
            # SBUF->SBUF DMAs. Zero quadrants memset once per buffer.
            # All units are WOVEN into the surrounding matmul stream:
            # PE transposes do NOT register as HAM clock-gate activity,
            # and neither do the tiny matmuls, so gaps must stay short.
            tw128s = [zp.tile([128, 4096], bf16, tag=f"tw{k}",
                              name=f"tw128_{k}") for k in range(3)]
            for k in range(3):
                nc.vector.memset(tw128s[k][0:64, 1728:4096], 0.0)
                nc.vector.memset(tw128s[k][64:128, 0:2048], 0.0)
                nc.vector.memset(tw128s[k][64:128, 3776:4096], 0.0)

            def twtile(w):
                return tw128s[w % 3]

            def marshal_units(w, with_x):
                g = gtiles[w]
                tw = twtile(w)
                twtB = tp.tile([64, 64 * NI], bf16, tag="tB",
                               name=f"twtB{w}")
                units = []
                if with_x:
                    # x as interaction feature 0 (cast f32 -> bf16);
                    # xe lives on both partition halves, so both copies
                    # are lane-aligned
                    units.append(lambda tw=tw, w=w: rcopy(
                        tw[0:64, 0:64],
                        xe[0:64, 128 * w:128 * w + 64].bitcast(f32)))
                    units.append(lambda tw=tw, w=w: rcopy(
                        tw[64:128, 2048:2112],
                        xe[64:128, 128 * w + 64:128 * (w + 1)].bitcast(f32)))
                for grp in range(4):
                    lo = 8 * grp
                    hi = min(lo + 8, NT)
                    pst = pt.tile([64, 1024], bf16, tag="tr",
                                  name=f"tr{w}{grp}")
                    for u in range(lo, hi):
                        units.append(
                            lambda pst=pst, u=u, lo=lo, g=g:
                            nc.tensor.transpose(
                                pst[:, 128 * (u - lo):128 * (u - lo + 1)],
                                g[:, 64 * u:64 * (u + 1)], ident[:]))
                    pstv = pst[:].rearrange("d (u s) -> d u s", s=128)
                    nu = hi - lo
                    # A-samples (sl 0-63) -> tw128 top half, in place
                    units.append(
                        lambda pstv=pstv, lo=lo, nu=nu, tw=tw:
                        rcopy(tw[0:64, 64 * (1 + lo):64 * (1 + lo + nu)
                                 ].rearrange("d (u p) -> d u p", p=64),
                              pstv[:, 0:nu, 0:64]))
                    # B-samples (sl 64-127) -> twtB staging
                    units.append(
                        lambda pstv=pstv, lo=lo, nu=nu, twtB=twtB:
                        rcopy(twtB[:, 64 * (1 + lo):64 * (1 + lo + nu)
                                   ].rearrange("d (u p) -> d u p", p=64),
                              pstv[:, 0:nu, 64:128]))
                    # partition-shift B into tw128 bottom half (covers
                    # the x slot too for grp 0)
                    blo = 64 * (1 + lo)
                    bhi = 64 * (1 + hi)
                    units.append(
                        lambda twtB=twtB, tw=tw, blo=blo, bhi=bhi:
                        nc.sync.dma_start(tw[64:128, 2048 + blo:2048 + bhi],
                                          twtB[:, blo:bhi]))
                return units, twtB

            u0, twtB0 = marshal_units(0, with_x=False)

            def drip(n):
                for _ in range(min(n, len(u0))):
                    u0.pop(0)()

            # ---- bottom MLP: h0 = relu(x @ bw0.T + bb0), wave-0
            # transposes woven between the matmuls ----
            h0 = hp.tile([128, 2048], f32r, tag="h0")
            for m in range(4):
                ps = pm.tile([128, BC], f32, tag="mlp")
                mmr(ps[:], w_bw0[:, 128 * m:128 * (m + 1)], xT[:],
                    start=True, stop=True)
                drip(2)
                nc.scalar.activation(h0[:, 512 * m:512 * (m + 1)], ps[:],
                                     RELU, bias=w_bb0[:, m:m + 1])
            # ---- h1 = relu(h0 @ bw1.T + bb1): K=512 (4 chunks), M=256 ----
            h1 = hp.tile([128, 1024], f32r, tag="h1")
            for n in range(2):
                ps = pm.tile([128, BC], f32, tag="mlp")
                for k in range(4):
                    mmr(ps[:], w_bw1[:, 256 * k + 128 * n:256 * k + 128 * (n + 1)],
                        h0[:, 512 * k:512 * (k + 1)],
                        start=(k == 0), stop=(k == 3))
                    drip(2)
                nc.scalar.activation(h1[:, 512 * n:512 * (n + 1)], ps[:],
                                     RELU, bias=w_bb1[:, n:n + 1])
            # ---- xe = h1 @ bw2.T + bb2: K=256 (2 chunks), M=128
            # (xe duplicated on both partition halves) ----
            xe = hp.tile([128, BC], f32r, tag="xe")
            psx = pm.tile([128, BC], f32, tag="mlp")
            for k in range(2):
                mmr(psx[:], w_bw2[:, 128 * k:128 * (k + 1)],
                    h1[:, 512 * k:512 * (k + 1)], start=(k == 0), stop=(k == 1))
                drip(3)
            drip(38)
            nc.scalar.activation(xe[:], psx[:], IDENT,
                                 bias=w_bb2[:, 0:1])
            if taps:
                nc.sync.dma_start(tap_d["dbg_xe"], xe[0:D, :].bitcast(f32))
            # wave-0 x feature (needs xe): both halves lane-aligned
            rcopy(twtile(0)[0:64, 0:64], xe[0:64, 0:64].bitcast(f32))
            rcopy(twtile(0)[64:128, 2048:2112],
                  xe[64:128, 64:128].bitcast(f32))

            # ---- pair-gram matmuls, per 128-sample wave: 64 matmuls of
            # [128,54]x[128,54] (2 samples each, K=128 block-diag), 2-up
            # PSUM-packed via 64-col tiling; next wave's marshalling
            # units woven 1-per-matmul. Pair p = samples (p, p+64); the
            # pair operand [128, (a 2: +1728, i 27: +64)] merges to a
            # single [64-stride, 54] free dim (BIR rhs constraint).
            zbuf = zp.tile([NI, 4 * 128 * NI], bf16, tag="zbuf")  # [27,13824]
            for w in range(NW):
                tw = twtile(w)
                v = tw[:].rearrange("d (a i p) -> d p a i", a=2, i=32)
                uq = (marshal_units(w + 1, with_x=True)[0]
                      if w + 1 < NW else [])
                for tau in range(2):
                    zq = pz.tile([128, 1024], f32, tag="z",
                                 name=f"zq{w}{tau}")
                    for half in range(2):
                        for q in range(16):
                            p = 32 * tau + 16 * half + q
                            op = v[:, p, :, :]  # [128, 2, 32] -> [128, 64]
                            mm(zq[64 * half:64 * half + 64,
                                  64 * q:64 * q + 64], op, op,
                               start=True, stop=True,
                               tile_position=(0, 64 * half))
                            for _ in range(2):
                                if uq:
                                    uq.pop(0)()
                    # drain the 4 valid diag-block sets (half t, a) to
                    # zbuf i-major: sample sl = 32tau + 16t + q + 64a,
                    # c = tau + 2a, kw = 16t + q; symmetric relabel puts
                    # j on partitions
                    for t in range(2):
                        for a in range(2):
                            c = tau + 2 * a
                            src = zq[64 * t + 32 * a:64 * t + 32 * a + NI,
                                     :].rearrange(
                                "j (q v2) -> j v2 q", v2=64)[
                                :, 32 * a:32 * a + NI, :]
                            dst = zbuf[
                                :, 3456 * c + 864 * w:3456 * c + 864 * (w + 1)
                            ].rearrange("j (i k) -> j i k", k=32)[
                                :, :, 16 * t:16 * t + 16]
                            rcopy(dst, src)
                while uq:
                    uq.pop(0)()

            if taps:
                nc.sync.dma_start(tap_d["dbg_zbuf"], zbuf[:])

            # keep the PE warm across the wave-3 drain tail, and overlap
            # it with the top-L1 x-part for m=0,1 (no z dependency) into
            # a freed gram-PSUM tile used as the L1 accumulator
            wdum = pm.tile([128, BC], f32, tag="mlp")
            for _ in range(30):
                mm(wdum[0:64, 0:128], ident[0:64, 0:64],
                   ident[0:64, 0:128], start=True, stop=True)
            xrhs = xe[0:64, :].rearrange("d (w c j) -> d c w j", c=4, j=32)
            pl01 = pz.tile([128, 1024], f32, tag="z")
            pl23 = pz.tile([128, 1024], f32, tag="z")
            for m in range(4):
                pl = (pl01, pl23)[m // 2][:, 512 * (m % 2):512 * (m % 2 + 1)]
                mmr(pl, w_t0x[:, 128 * m:128 * (m + 1)], xrhs,
                    start=True, stop=False)

            # ---- repack Z into 3 dense K-chunks of lower-tri pair rows:
            # row t = i(i-1)/2 + j - 128*gch. Selector matmuls on the PE:
            # sel[j, m] = (j < i and t == 128g + m), so each mm lands i's
            # valid j-rows at their pair positions, zero elsewhere; one
            # contiguous [128,512] f32->bf16 drain per chunk.
            # rhs view per i: [27(j), (c, w, kw)] -- 32-element (64B)
            # contiguous runs, full-rate PE streaming; N order (c, w, kw)
            # matches the zdense/top-L1 sample order 128c + 32w + kw
            zr = zbuf[:].rearrange("j (c w i k) -> j c w i k", c=4, w=NW,
                                   i=NI)

            def repack_chunk(gch):
                psr = pm.tile([128, BC], f32, tag="mlp", name=f"psr{gch}")
                sub = [(si, i) for si, (gc, i) in enumerate(plan)
                       if gc == gch]
                for nn, (si, i) in enumerate(sub):
                    mm(psr[:], selm[:, 128 * si:128 * (si + 1)],
                       zr[:, :, :, i, :],
                       start=(nn == 0), stop=(nn == len(sub) - 1))
                rcopy(zdense[gch][:], psr[:])

            def l1z(gch):
                for m in range(4):
                    pl = (pl01, pl23)[m // 2][
                        :, 512 * (m % 2):512 * (m % 2 + 1)]
                    mm(pl,
                       w_t0z[:, T0 * gch + 128 * m:T0 * gch + 128 * (m + 1)],
                       zdense[gch][:],
                       start=False, stop=(gch == NZC - 1))

            repack_chunk(0)
            repack_chunk(1)
            l1z(0)
            repack_chunk(2)
            l1z(1)
            l1z(2)

            if taps:
                for _g in range(NZC):
                    nc.sync.dma_start(
                        tap_d["dbg_zdense"].rearrange(
                            "p (g s) -> p g s", g=NZC)[:, _g, :],
                        zdense[_g][:])
            # ---- top L1 activations (x-parts + z accumulated above) ----
            o1 = hp.tile([128, 2048], f32r, tag="h0")
            for m in range(4):
                pl = (pl01, pl23)[m // 2][:, 512 * (m % 2):512 * (m % 2 + 1)]
                nc.scalar.activation(o1[:, 512 * m:512 * (m + 1)], pl,
                                     RELU, bias=w_tb0[:, m:m + 1])
            if taps:
                nc.sync.dma_start(tap_d["dbg_o1"], o1[:])
            # ---- top L2: K=512 (4 chunks), M=256 ----
            o2 = hp.tile([128, 1024], f32r, tag="h1")
            for n in range(2):
                ps = pm.tile([128, BC], f32, tag="mlp")
                for k in range(4):
                    mmr(ps[:], w_tw1[:, 256 * k + 128 * n:256 * k + 128 * (n + 1)],
                        o1[:, 512 * k:512 * (k + 1)],
                        start=(k == 0), stop=(k == 3))
                nc.scalar.activation(o2[:, 512 * n:512 * (n + 1)], ps[:],
                                     RELU, bias=w_tb1[:, n:n + 1])
            # ---- top L3: K=256 (2 chunks), M=1 ----
            osb = hp.tile([1, BC], f32, tag="osb")
            ps3 = pm.tile([128, BC], f32, tag="mlp")
            for k in range(2):
                mmr(ps3[0:1, :], w_tw2[:, k:k + 1],
                    o2[:, 512 * k:512 * (k + 1)], start=(k == 0), stop=(k == 1))
            nc.scalar.activation(osb[:], ps3[0:1, :], IDENT,
                                 bias=w_tb2[0:1, 0:1])
            nc.sync.dma_start(out_d, osb[:])
    nc.compile()
    return nc


def _host_prep(inputs):
    import ml_dtypes
    f = np.float32
    dense_x = np.asarray(inputs["dense_x"], f)
    sparse_idx = np.asarray(inputs["sparse_idx"])
    emb = np.ascontiguousarray(
        np.asarray(inputs["emb"], f).reshape(NT * V, D).astype(
            ml_dtypes.bfloat16))
    gl = (np.arange(NT, dtype=np.int64)[:, None] * V + sparse_idx).astype(
        np.int32)  # [26, 4096] global row ids

    bw0, bb0 = np.asarray(inputs["bw0"], f), np.asarray(inputs["bb0"], f)
    bw1, bb1 = np.asarray(inputs["bw1"], f), np.asarray(inputs["bb1"], f)
    bw2, bb2 = np.asarray(inputs["bw2"], f), np.asarray(inputs["bb2"], f)
    tw0, tb0 = np.asarray(inputs["tw0"], f), np.asarray(inputs["tb0"], f)
    tw1, tb1 = np.asarray(inputs["tw1"], f), np.asarray(inputs["tb1"], f)
    tw2, tb2 = np.asarray(inputs["tw2"], f), np.asarray(inputs["tb2"], f)

    def kpack(wT, nk, m):  # [K, M] -> [128, nk*m] chunk-major
        return np.ascontiguousarray(
            wT.reshape(nk, 128, m).transpose(1, 0, 2).reshape(128, nk * m))

    # top-L1 z weights: pair t = i(i-1)/2 + j (tril_indices order), one
    # row per lower-triangle pair, pad rows 351..383 zero
    wpad = np.zeros((NZC * 128, T0), f)
    wpad[:NP] = tw0[:, D:].T  # [351, 512]
    tw0z = kpack(wpad, NZC, T0).astype(ml_dtypes.bfloat16)

    # repack selectors: for (gch, i): sel[j, m] = 1 iff j < i and
    # i(i-1)/2 + j == 128*gch + m
    plan = _repack_plan()
    selm = np.zeros((NI, (len(plan) + 1) * 128), f)
    for si, (gch, i) in enumerate(plan):
        for j in range(i):
            t = i * (i - 1) // 2 + j
            m = t - 128 * gch
            if 0 <= m < 128:
                selm[j, 128 * si + m] = 1.0

    bb2p = np.zeros((128, 1), f)
    bb2p[:D, 0] = bb2
    bb2p[D:, 0] = bb2
    tb2p = np.zeros((128, 1), f)
    tb2p[0, 0] = tb2.reshape(1)[0]
    bw2d = np.ascontiguousarray(
        np.concatenate([bw2.T, bw2.T], axis=1))  # [256, 128]
    wblob = np.ascontiguousarray(np.concatenate([
        bb0.reshape(4, 128).T,                 # [128, 4]
        kpack(np.ascontiguousarray(bw1.T), 4, H1),   # [128, 1024]
        bb1.reshape(2, 128).T,                 # [128, 2]
        kpack(bw2d, 2, 128),                   # [128, 256]
        bb2p,                                  # [128, 1]
        tb0.reshape(4, 128).T,                 # [128, 4]
        kpack(np.ascontiguousarray(tw1.T), 4, T1),   # [128, 1024]
        tb1.reshape(2, 128).T,                 # [128, 2]
        kpack(np.ascontiguousarray(tw2.T), 2, 1),    # [128, 2]
        tb2p,                                  # [128, 1]
    ], axis=1))
    bw0T = np.ascontiguousarray(bw0.T)         # [13, 512]
    shared = {
        "emb": emb,
        "wblob": wblob,
        "tw0x": np.ascontiguousarray(tw0[:, :D].T),               # [64, 512]
        "tw0z": tw0z,
        "selm": selm.astype(ml_dtypes.bfloat16),
        "identc": np.eye(128, dtype=f).astype(ml_dtypes.bfloat16),
    }
    in_maps = []
    for c in range(NCORES):
        sl = gl[:, BC * c:BC * (c + 1)]  # [26, 512]
        offs = np.ascontiguousarray(
            sl.reshape(NT, NW, 128).transpose(2, 1, 0).reshape(128, NW * NT))
        m = dict(shared)
        m["offs"] = offs
        m["wb13"] = np.ascontiguousarray(np.concatenate(
            [bw0T, dense_x[BC * c:BC * (c + 1)].T], axis=1))
        in_maps.append(m)
    return in_maps


def _unpermute():
    s = np.arange(BC)
    pos = ((s % 128) // 32) * 128 + (s // 128) * 32 + (s % 32)
    return pos  # out[s] = outT[0, pos[s]]


def kernel(**inputs):
    from concourse import bass_utils
    if "nc" not in _CACHE:
        _CACHE["nc"] = _build_program()
    nc = _CACHE["nc"]
    in_maps = _host_prep(inputs)
    res = bass_utils.run_bass_kernel_spmd(nc, in_maps,
                                          core_ids=list(range(NCORES)))
    pos = _unpermute()
    out = np.empty((B, 1), np.float32)
    for c in range(NCORES):
        out[BC * c:BC * (c + 1), 0] = res.results[c]["outT"][0, pos]
    return out


# revision 28
# speedup vs baseline: 1.1933x; 1.0019x over previous
"""DLRM (bottom MLP + embedding gather + pairwise interaction + top MLP)
on 8 Trainium2 NeuronCores, batch-parallel (512 samples/core), embedding
tables replicated. All sharding/marshalling on host; one SPMD Bass program.

Numerics: dense path (bottom MLP, top MLP x-part/L2/L3) in float32r
matmuls (full-rate fp32, ~tf32 rounding, ~1e-4 rel); interaction path
(embedding gather, grams, top-L1 Z-part) in bf16.

Layout: embeddings gathered bf16 [sample-part, table*d]; PE-transposed
per table ([128,64]->[64,128], 8 sharing a [64,1024] PSUM tile drained
with one contiguous copy) into feature-major tw [64(d), feat*128]; per-
sample gram matmuls read strided [64, 27] views, 4-way PSUM-quadrant
packed; Z drained into j-partition zbuf i-major (contiguous writes,
so the repack rhs streams 64B runs at full PE rate); repack ON THE PE
via selector matmuls that compact the 351 lower-triangle pairs into 3
dense 128-row K-chunks (row t = i(i-1)/2 + j - 128g), so top-L1 does
only 12 z-matmuls and tw0z is 351 rows, with auto-zero pad rows.
The HAM clock gate needs ~3.4us sustained matmul activity for 2.4GHz
and PE-mode transposes do NOT count as activity, so the warm-up is
~3.6us, dummy matmuls are sprinkled between transpose groups, and the
wave-3-drain tail is bridged with dummies + top-L1 x-part matmuls.
"""
import numpy as np

B = 4096
NCORES = 8
BC = B // NCORES          # 512 samples per core
NT = 26                   # embedding tables
V = 100000                # vocab per table
D = 64                    # embedding dim
NI = NT + 1               # 27 interaction features
M_DEN = 13
H0, H1 = 512, 256         # bottom MLP hidden (13->512->256->64)
T0, T1 = 512, 256         # top MLP hidden (415->512->256->1)
NP = NI * (NI - 1) // 2   # 351 lower-triangle pairs
NZC = 3                   # dense K-chunks of 128 pair-rows (384 >= 351)
NW = 4                    # waves (one per 128-sample block)

_CACHE = {}


def _repack_plan():
    """(gch, i, start, stop) per selector matmul: chunk gch accumulates
    pair-rows t = i(i-1)/2 + j (j < i) that fall in [128g, 128(g+1))."""
    plan = []  # [(gch, i, slot)]
    for gch in range(NZC):
        lo, hi = 128 * gch, 128 * (gch + 1)
        for i in range(1, NI):
            t0, t1 = i * (i - 1) // 2, i * (i - 1) // 2 + i
            if t1 > lo and t0 < hi:
                plan.append((gch, i))
    return plan


def _build_program(taps=False):
    import concourse.bass as bass
    import concourse.bacc as bacc
    import concourse.mybir as mybir
    import concourse.tile as tile
    from concourse.masks import make_identity
    from contextlib import ExitStack

    dt = mybir.dt
    f32, bf16, i32 = dt.float32, dt.bfloat16, dt.int32
    f32r = dt.float32r  # fp32 @ 1cyc/col on PE (N>=256), ~tf32 rounding

    nc = bacc.Bacc("TRN2", target_bir_lowering=False, debug=False,
                   num_devices=NCORES)

    def din(name, shape, dtype=f32):
        return nc.dram_tensor(name, shape, dtype, kind="ExternalInput").ap()

    plan = _repack_plan()
    NSEL = len(plan)  # 28 selector matrices

    emb = din("emb", [NT * V, D], bf16)
    offs_d = din("offs", [128, NW * NT], i32)
    # merged const blobs (fewer DMAs):
    # wb13 = [bw0 | xT] on 13 partitions
    wb13_d = din("wb13", [M_DEN, H0 + BC], f32r)
    # wblob f32 [128, 2320]:
    #   bb0[0:4] bw1[4:1028] bb1[1028:1030] bw2x2[1030:1286] bb2[1286:1287]
    #   tb0[1287:1291] tw1[1291:2315] tb1[2315:2317] tw2[2317:2319]
    #   tb2 at [0, 2319]; bw2/bb2 are column-duplicated so the last
    #   bottom-MLP layer emits xe on BOTH partition halves (M=128) --
    #   the pair-gram B-half x feature then needs no partition-shift DMA
    wblob_d = din("wblob", [128, 2320], f32r)
    tw0x = din("tw0x", [D, T0], f32r)          # [64, 512]
    tw0z = din("tw0z", [128, NZC * T0], bf16)  # 3 dense K-chunks [128, 512]
    selm_d = din("selm", [NI, (NSEL + 1) * 128], bf16)  # repack sels + zero
    out_d = nc.dram_tensor("outT", [1, BC], f32, kind="ExternalOutput").ap()
    tap_d = {}
    if taps:
        for nm, shape, dty in [
                ("dbg_xe", [D, BC], f32), ("dbg_g0", [128, NT * D], bf16),
                ("dbg_tw0", [64, 64 * NI], bf16),
                ("dbg_zbuf", [NI, 4 * 128 * NI], bf16),
                ("dbg_zdense", [128, NZC * BC], bf16),
                ("dbg_o1", [128, 2048], f32)]:
            tap_d[nm] = nc.dram_tensor(nm, shape, dty,
                                       kind="ExternalOutput").ap()

    with tile.TileContext(nc) as tc:
        with ExitStack() as ctx:
            cp = ctx.enter_context(tc.tile_pool(name="const", bufs=1))
            gp = ctx.enter_context(tc.tile_pool(name="gath", bufs=4))
            tp = ctx.enter_context(tc.tile_pool(name="tall", bufs=3))
            zp = ctx.enter_context(tc.tile_pool(name="zbuf", bufs=1))
            hp = ctx.enter_context(tc.tile_pool(name="acts", bufs=1))
            pt = ctx.enter_context(
                tc.tile_pool(name="ps_t", bufs=2, space="PSUM"))
            pz = ctx.enter_context(
                tc.tile_pool(name="ps_z", bufs=2, space="PSUM"))
            pm = ctx.enter_context(
                tc.tile_pool(name="ps_m", bufs=2, space="PSUM"))

            def const_tile(ap, shape, tag=None):
                t = cp.tile(shape, ap.dtype, tag=tag or ap.tensor.name)
                nc.sync.dma_start(t[:], ap)
                return t

            offs = const_tile(offs_d, [128, NW * NT])
            wb13 = const_tile(wb13_d, [M_DEN, H0 + BC])
            ident = cp.tile([128, 128], bf16, tag="ident")
            make_identity(nc, ident[:])
            wblob = const_tile(wblob_d, [128, 2320])
            # tw0x/tw0z/selm are needed only after the wave loop; their
            # DMAs are emitted after wave-0 so the wave-0 partition-
            # shift DMAs aren't queued behind ~0.6MB on the HWDGE ring
            late_consts = {}
            w_bw0 = wb13[:, 0:H0]
            xT = wb13[:, H0:H0 + BC]
            w_bw1 = wblob[:, 4:1028]
            w_bw2 = wblob[:, 1030:1286]
            w_tw1 = wblob[:, 1291:2315]
            w_tw2 = wblob[:, 2317:2319]
            # biases: plain-f32 views of the f32r blob
            w_bb0 = wblob[:, 0:4].bitcast(f32)
            w_bb1 = wblob[:, 1028:1030].bitcast(f32)
            w_bb2 = wblob[:, 1286:1287].bitcast(f32)
            w_tb0 = wblob[:, 1287:1291].bitcast(f32)
            w_tb1 = wblob[:, 2315:2317].bitcast(f32)
            w_tb2 = wblob[0:1, 2319:2320].bitcast(f32)

            RELU = mybir.ActivationFunctionType.Relu
            IDENT = mybir.ActivationFunctionType.Identity
            mm = nc.tensor.matmul
            mmr = mm  # operands are f32r-typed tiles already

            # gpsimd copies are slow (~2us fixed) and cannot read PSUM:
            # all marshalling copies alternate vector/scalar
            _ps_engines = [nc.vector.tensor_copy, nc.scalar.copy]
            _ci = [0]

            def rcopy(dst, src):
                _ps_engines[_ci[0] % 2](dst, src)
                _ci[0] += 1

            # zdense: 3 K-chunk tiles, fully written by the selector-
            # matmul repack (pad rows come out zero), no memset needed
            zdense = [zp.tile([128, BC], bf16, tag=f"zd{g2}",
                              name=f"zd{g2}")
                      for g2 in range(NZC)]

            # PE warm-up: dummy matmuls on the identity. The gpsimd-
            # built ident is ready ~7us (DMA-fed data has a ~9us floor,
            # so this is the earliest the PE can start); the HAM gate
            # needs ~3.4us of sustained matmul activity.
            wps = pm.tile([128, BC], f32, tag="mlp")
            for _ in range(28):
                mm(wps[0:64, 0:128], ident[0:64, 0:64],
                   ident[0:64, 0:128], start=True, stop=True)

            # issue all 4 wave gathers up front (gpsimd DGE runs ahead)
            gtiles = []
            for w in range(NW):
                g = gp.tile([128, NT * D], bf16, tag="g")
                nc.gpsimd.indirect_dma_start(
                    out=g[:],
                    out_offset=None,
                    in_=emb,
                    in_offset=bass.IndirectOffsetOnAxis(
                        ap=offs[:, NT * w:NT * (w + 1)], axis=0),
                )
                gtiles.append(g)

            # ---- wave marshalling: quad-gram operand tiles ----
            # Grams are batched 4 samples per matmul: K=128 block-diag
            # (pair A-sample on partitions 0-63, B-sample on 64-127,
            # zero elsewhere) x M=108 (two pairs side by side; cross-
            # pair same-half products land in unused out positions).
            # tw128 layout [128, (a 2, i 27, p 64)]: col = 1728a+64i+p;
            # pair p = wave samples (p, p+64). A-halves drain straight
            # from the transpose PSUM (lane-aligned); B-halves stage in
            # twtB and shift partitions 0-63 -> 64-127 via per-group
            # SBUF->SBUF DMAs. Zero quadrants memset once per buffer.
            # All units are WOVEN into the surrounding matmul stream:
            # PE transposes do NOT register as HAM clock-gate activity,
            # and neither do the tiny matmuls, so gaps must stay short.
            tw128s = [zp.tile([128, 4096], bf16, tag=f"tw{k}",
                              name=f"tw128_{k}") for k in range(3)]
            for k in range(3):
                nc.vector.memset(tw128s[k][0:64, 1728:4096], 0.0)
                nc.vector.memset(tw128s[k][64:128, 0:2048], 0.0)
                nc.vector.memset(tw128s[k][64:128, 3776:4096], 0.0)

            def twtile(w):
                return tw128s[w % 3]

            def marshal_units(w, with_x):
                g = gtiles[w]
                tw = twtile(w)
                twtB = tp.tile([64, 64 * NI], bf16, tag="tB",
                               name=f"twtB{w}")
                units = []
                if with_x:
                    # x as interaction feature 0 (cast f32 -> bf16);
                    # xe lives on both partition halves, so both copies
                    # are lane-aligned
                    units.append(lambda tw=tw, w=w: rcopy(
                        tw[0:64, 0:64],
                        xe[0:64, 128 * w:128 * w + 64].bitcast(f32)))
                    units.append(lambda tw=tw, w=w: rcopy(
                        tw[64:128, 2048:2112],
                        xe[64:128, 128 * w + 64:128 * (w + 1)].bitcast(f32)))
                for grp in range(4):
                    lo = 8 * grp
                    hi = min(lo + 8, NT)
                    pst = pt.tile([64, 1024], bf16, tag="tr",
                                  name=f"tr{w}{grp}")
                    for u in range(lo, hi):
                        units.append(
                            lambda pst=pst, u=u, lo=lo, g=g:
                            nc.tensor.transpose(
                                pst[:, 128 * (u - lo):128 * (u - lo + 1)],
                                g[:, 64 * u:64 * (u + 1)], ident[:]))
                    pstv = pst[:].rearrange("d (u s) -> d u s", s=128)
                    nu = hi - lo
                    # A-samples (sl 0-63) -> tw128 top half, in place
                    units.append(
                        lambda pstv=pstv, lo=lo, nu=nu, tw=tw:
                        rcopy(tw[0:64, 64 * (1 + lo):64 * (1 + lo + nu)
                                 ].rearrange("d (u p) -> d u p", p=64),
                              pstv[:, 0:nu, 0:64]))
                    # B-samples (sl 64-127) -> twtB staging
                    units.append(
                        lambda pstv=pstv, lo=lo, nu=nu, twtB=twtB:
                        rcopy(twtB[:, 64 * (1 + lo):64 * (1 + lo + nu)
                                   ].rearrange("d (u p) -> d u p", p=64),
                              pstv[:, 0:nu, 64:128]))
                    # partition-shift B into tw128 bottom half (covers
                    # the x slot too for grp 0)
                    blo = 64 * (1 + lo)
                    bhi = 64 * (1 + hi)
                    units.append(
                        lambda twtB=twtB, tw=tw, blo=blo, bhi=bhi:
                        nc.sync.dma_start(tw[64:128, 2048 + blo:2048 + bhi],
                                          twtB[:, blo:bhi]))
                return units, twtB

            u0, twtB0 = marshal_units(0, with_x=False)

            def drip(n):
                for _ in range(min(n, len(u0))):
                    u0.pop(0)()

            # ---- bottom MLP: h0 = relu(x @ bw0.T + bb0), wave-0
            # transposes woven between the matmuls ----
            h0 = hp.tile([128, 2048], f32r, tag="h0")
            for m in range(4):
                ps = pm.tile([128, BC], f32, tag="mlp")
                mmr(ps[:], w_bw0[:, 128 * m:128 * (m + 1)], xT[:],
                    start=True, stop=True)
                drip(2)
                nc.scalar.activation(h0[:, 512 * m:512 * (m + 1)], ps[:],
                                     RELU, bias=w_bb0[:, m:m + 1])
            # ---- h1 = relu(h0 @ bw1.T + bb1): K=512 (4 chunks), M=256 ----
            h1 = hp.tile([128, 1024], f32r, tag="h1")
            for n in range(2):
                ps = pm.tile([128, BC], f32, tag="mlp")
                for k in range(4):
                    mmr(ps[:], w_bw1[:, 256 * k + 128 * n:256 * k + 128 * (n + 1)],
                        h0[:, 512 * k:512 * (k + 1)],
                        start=(k == 0), stop=(k == 3))
                    drip(2)
                nc.scalar.activation(h1[:, 512 * n:512 * (n + 1)], ps[:],
                                     RELU, bias=w_bb1[:, n:n + 1])
            # ---- xe = h1 @ bw2.T + bb2: K=256 (2 chunks), M=128
            # (xe duplicated on both partition halves) ----
            xe = hp.tile([128, BC], f32r, tag="xe")
            psx = pm.tile([128, BC], f32, tag="mlp")
            for k in range(2):
                mmr(psx[:], w_bw2[:, 128 * k:128 * (k + 1)],
                    h1[:, 512 * k:512 * (k + 1)], start=(k == 0), stop=(k == 1))
                drip(3)
            drip(38)
            nc.scalar.activation(xe[:], psx[:], IDENT,
                                 bias=w_bb2[:, 0:1])
            if taps:
                nc.sync.dma_start(tap_d["dbg_xe"], xe[0:D, :].bitcast(f32))
            # wave-0 x feature (needs xe): both halves lane-aligned
            rcopy(twtile(0)[0:64, 0:64], xe[0:64, 0:64].bitcast(f32))
            rcopy(twtile(0)[64:128, 2048:2112],
                  xe[64:128, 64:128].bitcast(f32))

            # ---- pair-gram matmuls, per 128-sample wave: 64 matmuls of
            # [128,54]x[128,54] (2 samples each, K=128 block-diag), 2-up
            # PSUM-packed via 64-col tiling; next wave's marshalling
            # units woven 1-per-matmul. Pair p = samples (p, p+64); the
            # pair operand [128, (a 2: +1728, i 27: +64)] merges to a
            # single [64-stride, 54] free dim (BIR rhs constraint).
            zbuf = zp.tile([NI, 4 * 128 * NI], bf16, tag="zbuf")  # [27,13824]
            for w in range(NW):
                tw = twtile(w)
                v = tw[:].rearrange("d (a i p) -> d p a i", a=2, i=32)
                uq = (marshal_units(w + 1, with_x=True)[0]
                      if w + 1 < NW else [])
                for tau in range(2):
                    zq = pz.tile([128, 1024], f32, tag="z",
                                 name=f"zq{w}{tau}")
                    for half in range(2):
                        for q in range(16):
                            p = 32 * tau + 16 * half + q
                            op = v[:, p, :, :]  # [128, 2, 32] -> [128, 64]
                            mm(zq[64 * half:64 * half + 64,
                                  64 * q:64 * q + 64], op, op,
                               start=True, stop=True,
                               tile_position=(0, 64 * half))
                            for _ in range(2):
                                if uq:
                                    uq.pop(0)()
                    # drain the 4 valid diag-block sets (half t, a) to
                    # zbuf i-major: sample sl = 32tau + 16t + q + 64a,
                    # c = tau + 2a, kw = 16t + q; symmetric relabel puts
                    # j on partitions
                    for t in range(2):
                        for a in range(2):
                            c = tau + 2 * a
                            src = zq[64 * t + 32 * a:64 * t + 32 * a + NI,
                                     :].rearrange(
                                "j (q v2) -> j v2 q", v2=64)[
                                :, 32 * a:32 * a + NI, :]
                            dst = zbuf[
                                :, 3456 * c + 864 * w:3456 * c + 864 * (w + 1)
                            ].rearrange("j (i k) -> j i k", k=32)[
                                :, :, 16 * t:16 * t + 16]
                            rcopy(dst, src)
                while uq:
                    uq.pop(0)()
                if w == 0:
                    late_consts["t0x"] = const_tile(tw0x, [D, T0])
                    late_consts["t0z"] = const_tile(tw0z,
                                                    [128, NZC * T0])
                    late_consts["selm"] = const_tile(
                        selm_d, [NI, (NSEL + 1) * 128])

            if taps:
                nc.sync.dma_start(tap_d["dbg_zbuf"], zbuf[:])

            # keep the PE warm across the wave-3 drain tail, and overlap
            # it with the top-L1 x-part for m=0,1 (no z dependency) into
            # a freed gram-PSUM tile used as the L1 accumulator
            wdum = pm.tile([128, BC], f32, tag="mlp")
            for _ in range(44):
                mm(wdum[0:64, 0:128], ident[0:64, 0:64],
                   ident[0:64, 0:128], start=True, stop=True)
            w_t0x = late_consts["t0x"]
            w_t0z = late_consts["t0z"]
            selm = late_consts["selm"]
            xrhs = xe[0:64, :].rearrange("d (w c j) -> d c w j", c=4, j=32)
            pl01 = pz.tile([128, 1024], f32, tag="z")
            pl23 = pz.tile([128, 1024], f32, tag="z")
            for m in range(4):
                pl = (pl01, pl23)[m // 2][:, 512 * (m % 2):512 * (m % 2 + 1)]
                mmr(pl, w_t0x[:, 128 * m:128 * (m + 1)], xrhs,
                    start=True, stop=False)

            # ---- repack Z into 3 dense K-chunks of lower-tri pair rows:
            # row t = i(i-1)/2 + j - 128*gch. Selector matmuls on the PE:
            # sel[j, m] = (j < i and t == 128g + m), so each mm lands i's
            # valid j-rows at their pair positions, zero elsewhere; one
            # contiguous [128,512] f32->bf16 drain per chunk.
            # rhs view per i: [27(j), (c, w, kw)] -- 32-element (64B)
            # contiguous runs, full-rate PE streaming; N order (c, w, kw)
            # matches the zdense/top-L1 sample order 128c + 32w + kw
            zr = zbuf[:].rearrange("j (c w i k) -> j c w i k", c=4, w=NW,
                                   i=NI)

            def repack_chunk(gch):
                psr = pm.tile([128, BC], f32, tag="mlp", name=f"psr{gch}")
                sub = [(si, i) for si, (gc, i) in enumerate(plan)
                       if gc == gch]
                for nn, (si, i) in enumerate(sub):
                    mm(psr[:], selm[:, 128 * si:128 * (si + 1)],
                       zr[:, :, :, i, :],
                       start=(nn == 0), stop=(nn == len(sub) - 1))
                rcopy(zdense[gch][:], psr[:])

            def l1z(gch):
                for m in range(4):
                    pl = (pl01, pl23)[m // 2][
                        :, 512 * (m % 2):512 * (m % 2 + 1)]
                    mm(pl,
                       w_t0z[:, T0 * gch + 128 * m:T0 * gch + 128 * (m + 1)],
                       zdense[gch][:],
                       start=False, stop=(gch == NZC - 1))

            repack_chunk(0)
            repack_chunk(1)
            repack_chunk(2)
            l1z(0)
            l1z(1)
            l1z(2)

            if taps:
                for _g in range(NZC):
                    nc.sync.dma_start(
                        tap_d["dbg_zdense"].rearrange(
                            "p (g s) -> p g s", g=NZC)[:, _g, :],
                        zdense[_g][:])
            # ---- top L1 activations (x-parts + z accumulated above) ----
            o1 = hp.tile([128, 2048], f32r, tag="h0")
            for m in range(4):
                pl = (pl01, pl23)[m // 2][:, 512 * (m % 2):512 * (m % 2 + 1)]
                nc.scalar.activation(o1[:, 512 * m:512 * (m + 1)], pl,
                                     RELU, bias=w_tb0[:, m:m + 1])
            if taps:
                nc.sync.dma_start(tap_d["dbg_o1"], o1[:])
            # ---- top L2: K=512 (4 chunks), M=256 ----
            o2 = hp.tile([128, 1024], f32r, tag="h1")
            for n in range(2):
                ps = pm.tile([128, BC], f32, tag="mlp")
                for k in range(4):
                    mmr(ps[:], w_tw1[:, 256 * k + 128 * n:256 * k + 128 * (n + 1)],
                        o1[:, 512 * k:512 * (k + 1)],
                        start=(k == 0), stop=(k == 3))
                nc.scalar.activation(o2[:, 512 * n:512 * (n + 1)], ps[:],
                                     RELU, bias=w_tb1[:, n:n + 1])
            # ---- top L3: K=256 (2 chunks), M=1 ----
            osb = hp.tile([1, BC], f32, tag="osb")
            ps3 = pm.tile([128, BC], f32, tag="mlp")
            for k in range(2):
                mmr(ps3[0:1, :], w_tw2[:, k:k + 1],
                    o2[:, 512 * k:512 * (k + 1)], start=(k == 0), stop=(k == 1))
            nc.scalar.activation(osb[:], ps3[0:1, :], IDENT,
                                 bias=w_tb2[0:1, 0:1])
            nc.sync.dma_start(out_d, osb[:])
    nc.compile()
    return nc


def _host_prep(inputs):
    import ml_dtypes
    f = np.float32
    dense_x = np.asarray(inputs["dense_x"], f)
    sparse_idx = np.asarray(inputs["sparse_idx"])
    emb = np.ascontiguousarray(
        np.asarray(inputs["emb"], f).reshape(NT * V, D).astype(
            ml_dtypes.bfloat16))
    gl = (np.arange(NT, dtype=np.int64)[:, None] * V + sparse_idx).astype(
        np.int32)  # [26, 4096] global row ids

    bw0, bb0 = np.asarray(inputs["bw0"], f), np.asarray(inputs["bb0"], f)
    bw1, bb1 = np.asarray(inputs["bw1"], f), np.asarray(inputs["bb1"], f)
    bw2, bb2 = np.asarray(inputs["bw2"], f), np.asarray(inputs["bb2"], f)
    tw0, tb0 = np.asarray(inputs["tw0"], f), np.asarray(inputs["tb0"], f)
    tw1, tb1 = np.asarray(inputs["tw1"], f), np.asarray(inputs["tb1"], f)
    tw2, tb2 = np.asarray(inputs["tw2"], f), np.asarray(inputs["tb2"], f)

    def kpack(wT, nk, m):  # [K, M] -> [128, nk*m] chunk-major
        return np.ascontiguousarray(
            wT.reshape(nk, 128, m).transpose(1, 0, 2).reshape(128, nk * m))

    # top-L1 z weights: pair t = i(i-1)/2 + j (tril_indices order), one
    # row per lower-triangle pair, pad rows 351..383 zero
    wpad = np.zeros((NZC * 128, T0), f)
    wpad[:NP] = tw0[:, D:].T  # [351, 512]
    tw0z = kpack(wpad, NZC, T0).astype(ml_dtypes.bfloat16)

    # repack selectors: for (gch, i): sel[j, m] = 1 iff j < i and
    # i(i-1)/2 + j == 128*gch + m
    plan = _repack_plan()
    selm = np.zeros((NI, (len(plan) + 1) * 128), f)
    for si, (gch, i) in enumerate(plan):
        for j in range(i):
            t = i * (i - 1) // 2 + j
            m = t - 128 * gch
            if 0 <= m < 128:
                selm[j, 128 * si + m] = 1.0

    bb2p = np.zeros((128, 1), f)
    bb2p[:D, 0] = bb2
    bb2p[D:, 0] = bb2
    tb2p = np.zeros((128, 1), f)
    tb2p[0, 0] = tb2.reshape(1)[0]
    bw2d = np.ascontiguousarray(
        np.concatenate([bw2.T, bw2.T], axis=1))  # [256, 128]
    wblob = np.ascontiguousarray(np.concatenate([
        bb0.reshape(4, 128).T,                 # [128, 4]
        kpack(np.ascontiguousarray(bw1.T), 4, H1),   # [128, 1024]
        bb1.reshape(2, 128).T,                 # [128, 2]
        kpack(bw2d, 2, 128),                   # [128, 256]
        bb2p,                                  # [128, 1]
        tb0.reshape(4, 128).T,                 # [128, 4]
        kpack(np.ascontiguousarray(tw1.T), 4, T1),   # [128, 1024]
        tb1.reshape(2, 128).T,                 # [128, 2]
        kpack(np.ascontiguousarray(tw2.T), 2, 1),    # [128, 2]
        tb2p,                                  # [128, 1]
    ], axis=1))
    bw0T = np.ascontiguousarray(bw0.T)         # [13, 512]
    shared = {
        "emb": emb,
        "wblob": wblob,
        "tw0x": np.ascontiguousarray(tw0[:, :D].T),               # [64, 512]
        "tw0z": tw0z,
        "selm": selm.astype(ml_dtypes.bfloat16),
    }
    in_maps = []
    for c in range(NCORES):
        sl = gl[:, BC * c:BC * (c + 1)]  # [26, 512]
        offs = np.ascontiguousarray(
            sl.reshape(NT, NW, 128).transpose(2, 1, 0).reshape(128, NW * NT))
        m = dict(shared)
        m["offs"] = offs
        m["wb13"] = np.ascontiguousarray(np.concatenate(
            [bw0T, dense_x[BC * c:BC * (c + 1)].T], axis=1))
        in_maps.append(m)
    return in_maps


def _unpermute():
    s = np.arange(BC)
    pos = ((s % 128) // 32) * 128 + (s // 128) * 32 + (s % 32)
    return pos  # out[s] = outT[0, pos[s]]


def kernel(**inputs):
    from concourse import bass_utils
    if "nc" not in _CACHE:
        _CACHE["nc"] = _build_program()
    nc = _CACHE["nc"]
    in_maps = _host_prep(inputs)
    res = bass_utils.run_bass_kernel_spmd(nc, in_maps,
                                          core_ids=list(range(NCORES)))
    pos = _unpermute()
    out = np.empty((B, 1), np.float32)
    for c in range(NCORES):
        out[BC * c:BC * (c + 1), 0] = res.results[c]["outT"][0, pos]
    return out
